# revision 42
# baseline (speedup 1.0000x reference)
"""Trainium2 Bass kernel: DiscreteEmbedding (rect-window embedding lookup).

Math (matches the jax reference):
    xs  = x * 2048;  y = xs + 0.5
    i_lo = ceil(y)-1, i_hi = floor(y)
    out[t] = 0.5*T[i_lo] + 0.5*T[i_hi]      (T extended with zero row 2048)
Non-boundary tokens (y non-integer): i_lo == i_hi -> out = T[i_lo].
Boundary tokens (y integer, ~1/8192 of tokens): out = avg of two rows.

Device strategy (8 cores, data-parallel over tokens):
  - Combined table TC built on the HOST (depends only on the weights):
      TC[0:2048] = T;  TC[2048] = 0;  TC[2049+k] = (T[k]+T[k+1])/2
    stored as bf16 bit patterns in uint16 (the gather is a pure byte
    mover; bf16 halves both gather-read and store HBM traffic and keeps
    rel err ~1.7e-3). One gather per token at idx2 = i_lo + 2049*b,
    b = (y integer) — this avoids a second gather for boundary tokens.
  - Gather via SWDGE dma_gather on all 4 queues. The s2m descriptor
    generator runs at a hard ~8 ns/desc per queue pair (measured; the
    m2s side and prepare_only mode are ~10x faster but the s2m side
    paces the op), so the gather wall is ~2048 descs x 8 ns ~ 17 us.
    Queue-0 ops block the POOL NX for their whole desc-gen; queues 1-3
    dispatch async, so every round dispatches q1..q3 first and q0 last.
    3 chunks per queue minimize per-chunk fixed overhead while keeping
    the drains and stores pipelined.
  - x is passed wrapped [16,512] replicated to [128,512]: full-width DVE
    index math, and partitions 16..127 double as the per-Q7-core replicas
    of the int16 index buffer that dma_gather expects. The x load and the
    index math fully overlap the fixed ~9.5us Q7 library IRAM load
    (the first extended-inst op cannot start before ~16.5 us).
  - No warm-up ops: the first chunk per queue pays its ~1.3us init
    inline at library-ready, concurrently across the 4 queue pairs,
    which measures slightly faster than a separate warm-up round.
  - Stores alternate between the SP (sync) and ACT (scalar) HWDGE rings;
    host un-permutes rows (free) while un-sharding and widens bf16->f32.
"""

import numpy as np

import concourse.mybir as mybir
import concourse.tile as tile
from concourse import bacc, bass_utils

N_CORES = 8
B, S = 32, 2048
V, D = 2048, 128
TOK = B * S                 # 65536 tokens total
TPC = TOK // N_CORES        # 8192 tokens per core
SPC = TPC // 16             # 512: free dim of the wrapped [16, 512] x layout
ABASE = V + 1               # 2049: base row of the averaged-pair table
VEXT = 4224                 # TC rows (>= 2*V+1, multiple of 128)
NQ = 4                      # SWDGE queues

# (j_block_start, j_block_count, queue) per chunk, in dispatch order.
# Even 16 j-blocks per queue in a (7,6,3) descending profile: big early
# chunks amortize per-chunk overhead while stores ramp up; the small
# final chunks complete (and release their stores) quickly at the end.
CHUNKS = [
    (0, 7, 1), (7, 7, 2), (14, 7, 3), (21, 7, 0),
    (28, 6, 1), (34, 6, 2), (40, 6, 3), (46, 6, 0),
    (52, 3, 1), (55, 3, 2), (58, 3, 3), (61, 3, 0),
]
assert sum(c[1] for c in CHUNKS) == TPC // 128
for _q in range(4):
    assert sum(c[1] for c in CHUNKS if c[2] == _q) == 16

F32 = mybir.dt.float32
I32 = mybir.dt.int32
I16 = mybir.dt.int16
U16 = mybir.dt.uint16
OP = mybir.AluOpType


def build():
    nc = bacc.Bacc(
        "TRN2",
        target_bir_lowering=False,
        debug=False,
        num_devices=N_CORES,
        num_swdge_queues=NQ,
    )
    xr = nc.dram_tensor("xr", [128, SPC], F32, kind="ExternalInput")
    tcb = nc.dram_tensor("tcb", [VEXT, D], U16, kind="ExternalInput")
    out = nc.dram_tensor("out", [TPC, D], U16, kind="ExternalOutput")

    with tile.TileContext(nc) as tc:
        with tc.tile_pool(name="sb", bufs=1) as sb, tc.tile_pool(name="g", bufs=1) as gp:
            nidx_regs = {
                n: nc.gpsimd.to_reg(128 * n) for n in sorted({c[1] for c in CHUNKS})
            }

            xt = sb.tile([128, SPC], F32)
            nc.sync.dma_start(out=xt[:], in_=xr[:])

            # ---- index math (fp32, exact): y = x*2048 + 0.5; i0 = rne(y);
            # fix up to i_lo = ceil(y)-1 with exact fp32 compares (robust to
            # the HW float->int round mode); idx2 = i_lo + 2049*(y integer).
            y = sb.tile([128, SPC], F32)
            nc.vector.tensor_scalar(y[:], xt[:], 2048.0, 0.5, op0=OP.mult, op1=OP.add)
            i0 = sb.tile([128, SPC], I32)
            nc.vector.tensor_copy(i0[:], y[:])
            f0 = sb.tile([128, SPC], F32)
            nc.vector.tensor_copy(f0[:], i0[:])
            lt = sb.tile([128, SPC], F32)
            nc.vector.tensor_tensor(lt[:], f0[:], y[:], op=OP.is_lt)
            bnd = sb.tile([128, SPC], F32)
            nc.vector.tensor_tensor(bnd[:], f0[:], y[:], op=OP.is_equal)
            lf = sb.tile([128, SPC], F32)
            nc.vector.scalar_tensor_tensor(
                out=lf[:], in0=f0[:], scalar=-1.0, in1=lt[:], op0=OP.add, op1=OP.add
            )
            idx16 = sb.tile([128, SPC], I16)
            nc.vector.scalar_tensor_tensor(
                out=idx16[:], in0=bnd[:], scalar=float(ABASE), in1=lf[:],
                op0=OP.mult, op1=OP.add,
            )

            # ---- chunked gather + store ----
            # Rounds 1-2: adjacent chunk PAIRS share a tile and one store
            # (their gens finish within ~0.3us of each other, so the merged
            # store's max-sem wait is hidden mid-stream; 4 fewer DMA
            # instructions shorten the teardown sem chain). Round 3 keeps
            # per-chunk stores — its waits sit on the latency tail.
            out_v = out[:].rearrange("(p j) d -> p (j d)", p=128)
            pair = {}
            for ci, (j0, jbc, q) in enumerate(CHUNKS):
                if ci < 8:
                    pi = ci // 2
                    if ci % 2 == 0:
                        pjbc = jbc + CHUNKS[ci + 1][1]
                        pt = gp.tile([128, pjbc * D], U16, tag=f"p{pi}")
                        pair[pi] = (pt, j0, pjbc)
                    pt, pj0, pjbc = pair[pi]
                    gv = pt[:, (j0 - pj0) * D : (j0 - pj0 + jbc) * D]
                else:
                    g = gp.tile([128, jbc * D], U16, tag=f"g{ci}")
                    gv = g[:]
                nc.gpsimd.dma_gather(
                    gv.rearrange("p (j d) -> p j d", d=D),
                    tcb[0 : ABASE + V],
                    idx16[:, j0 * 8 : (j0 + jbc) * 8],
                    num_idxs=128 * jbc,
                    num_idxs_reg=nidx_regs[jbc],
                    elem_size=D,
                    single_packet=False,
                    queue_num=q,
                )
                if ci < 8 and ci % 2 == 1:
                    pt, pj0, pjbc = pair[ci // 2]
                    eng = nc.sync if (ci // 2) % 2 == 0 else nc.scalar
                    eng.dma_start(
                        out=out_v[:, pj0 * D : (pj0 + pjbc) * D], in_=pt[:]
                    )
                elif ci >= 8:
                    eng = nc.sync if ci % 2 == 0 else nc.scalar
                    eng.dma_start(out=out_v[:, j0 * D : (j0 + jbc) * D], in_=gv)
    nc.compile()
    return nc


_NC = None


def _row_perm():
    """out row r holds gather position i(r); position i handles token
    t(i) = (i%16)*512 + i//16 (x wrapped [16,512] across partitions)."""
    r = np.arange(TPC)
    p, j = r // 64, r % 64
    i = j * 128 + p
    return (i % 16) * SPC + i // 16  # token index held at row r


def _f32_to_bf16_bits(a):
    bits = np.ascontiguousarray(a, dtype=np.float32).view(np.uint32)
    return (((bits + 0x7FFF + ((bits >> 16) & 1)) >> 16) & 0xFFFF).astype(np.uint16)


def _build_tc(t):
    tc = np.zeros((VEXT, D), dtype=np.float32)
    tc[0:V] = t
    ext = np.vstack([t, np.zeros((1, D), dtype=np.float32)])
    tc[ABASE : ABASE + V] = 0.5 * (ext[0:V] + ext[1 : V + 1])
    return _f32_to_bf16_bits(tc)


def kernel(x, time_embedding):
    global _NC
    x = np.ascontiguousarray(np.asarray(x, dtype=np.float32))
    t = np.ascontiguousarray(np.asarray(time_embedding, dtype=np.float32))
    tcb = _build_tc(t)
    xf = x.reshape(-1)
    in_maps = []
    for c in range(N_CORES):
        xc = xf[c * TPC : (c + 1) * TPC].reshape(16, SPC)
        in_maps.append({"xr": np.ascontiguousarray(np.tile(xc, (8, 1))), "tcb": tcb})

    if _NC is None:
        _NC = build()
    res = bass_utils.run_bass_kernel_spmd(_NC, in_maps, core_ids=list(range(N_CORES)))
    global _LAST_RES
    _LAST_RES = res

    tkn = _row_perm()
    outs = []
    for c in range(N_CORES):
        oc = np.asarray(res.results[c]["out"])  # [TPC, D] uint16 (bf16 bits)
        of = (oc.astype(np.uint32) << 16).view(np.float32)
        full = np.empty_like(of)
        full[tkn] = of
        outs.append(full)
    return np.concatenate(outs, axis=0).reshape(B, S, D)


# revision 43
# speedup vs baseline: 1.0083x; 1.0083x over previous
"""Trainium2 Bass kernel: DiscreteEmbedding (rect-window embedding lookup).

Math (matches the jax reference):
    xs  = x * 2048;  y = xs + 0.5
    i_lo = ceil(y)-1, i_hi = floor(y)
    out[t] = 0.5*T[i_lo] + 0.5*T[i_hi]      (T extended with zero row 2048)
Non-boundary tokens (y non-integer): i_lo == i_hi -> out = T[i_lo].
Boundary tokens (y integer, ~1/8192 of tokens): out = avg of two rows.

Device strategy (8 cores, data-parallel over tokens):
  - Combined table TC built on the HOST (depends only on the weights):
      TC[0:2048] = T;  TC[2048] = 0;  TC[2049+k] = (T[k]+T[k+1])/2
    stored as bf16 bit patterns in uint16 (the gather is a pure byte
    mover; bf16 halves both gather-read and store HBM traffic and keeps
    rel err ~1.7e-3). One gather per token at idx2 = i_lo + 2049*b,
    b = (y integer) — this avoids a second gather for boundary tokens.
  - Gather via SWDGE dma_gather on all 4 queues. The s2m descriptor
    generator runs at a hard ~8 ns/desc per queue pair (measured; the
    m2s side and prepare_only mode are ~10x faster but the s2m side
    paces the op), so the gather wall is ~2048 descs x 8 ns ~ 17 us.
    Queue-0 ops block the POOL NX for their whole desc-gen; queues 1-3
    dispatch async, so every round dispatches q1..q3 first and q0 last.
    3 chunks per queue minimize per-chunk fixed overhead while keeping
    the drains and stores pipelined.
  - x is passed wrapped [16,512] replicated to [128,512]: full-width DVE
    index math, and partitions 16..127 double as the per-Q7-core replicas
    of the int16 index buffer that dma_gather expects. The x load and the
    index math fully overlap the fixed ~9.5us Q7 library IRAM load
    (the first extended-inst op cannot start before ~16.5 us).
  - No warm-up ops: the first chunk per queue pays its ~1.3us init
    inline at library-ready, concurrently across the 4 queue pairs,
    which measures slightly faster than a separate warm-up round.
  - Stores alternate between the SP (sync) and ACT (scalar) HWDGE rings;
    host un-permutes rows (free) while un-sharding and widens bf16->f32.
"""

import numpy as np

import concourse.mybir as mybir
import concourse.tile as tile
from concourse import bacc, bass_utils

N_CORES = 8
B, S = 32, 2048
V, D = 2048, 128
TOK = B * S                 # 65536 tokens total
TPC = TOK // N_CORES        # 8192 tokens per core
SPC = TPC // 16             # 512: free dim of the wrapped [16, 512] x layout
ABASE = V + 1               # 2049: base row of the averaged-pair table
VEXT = 4224                 # TC rows (>= 2*V+1, multiple of 128)
NQ = 4                      # SWDGE queues

# (j_block_start, j_block_count, queue) per chunk, in dispatch order.
# Even 16 j-blocks per queue in a (7,6,3) descending profile: big early
# chunks amortize per-chunk overhead while stores ramp up; the small
# final chunks complete (and release their stores) quickly at the end.
CHUNKS = [
    (0, 7, 1), (7, 7, 2), (14, 7, 3), (21, 7, 0),
    (28, 6, 1), (34, 6, 2), (40, 6, 3), (46, 6, 0),
    (52, 3, 1), (55, 3, 2), (58, 3, 3), (61, 3, 0),
]
assert sum(c[1] for c in CHUNKS) == TPC // 128
for _q in range(4):
    assert sum(c[1] for c in CHUNKS if c[2] == _q) == 16

F32 = mybir.dt.float32
I32 = mybir.dt.int32
I16 = mybir.dt.int16
U16 = mybir.dt.uint16
OP = mybir.AluOpType


def build():
    nc = bacc.Bacc(
        "TRN2",
        target_bir_lowering=False,
        debug=False,
        num_devices=N_CORES,
        num_swdge_queues=NQ,
    )
    xr = nc.dram_tensor("xr", [128, SPC], F32, kind="ExternalInput")
    tcb = nc.dram_tensor("tcb", [VEXT, D], U16, kind="ExternalInput")
    out = nc.dram_tensor("out", [TPC, D], U16, kind="ExternalOutput")

    with tile.TileContext(nc) as tc:
        with tc.tile_pool(name="sb", bufs=1) as sb, tc.tile_pool(name="g", bufs=1) as gp:
            nidx_regs = {
                n: nc.gpsimd.to_reg(128 * n) for n in sorted({c[1] for c in CHUNKS})
            }

            xt = sb.tile([128, SPC], F32)
            nc.sync.dma_start(out=xt[:], in_=xr[:])

            # ---- index math (fp32, exact): y = x*2048 + 0.5; i0 = rne(y);
            # fix up to i_lo = ceil(y)-1 with exact fp32 compares (robust to
            # the HW float->int round mode); idx2 = i_lo + 2049*(y integer).
            y = sb.tile([128, SPC], F32)
            nc.vector.tensor_scalar(y[:], xt[:], 2048.0, 0.5, op0=OP.mult, op1=OP.add)
            i0 = sb.tile([128, SPC], I32)
            nc.vector.tensor_copy(i0[:], y[:])
            f0 = sb.tile([128, SPC], F32)
            nc.vector.tensor_copy(f0[:], i0[:])
            lt = sb.tile([128, SPC], F32)
            nc.vector.tensor_tensor(lt[:], f0[:], y[:], op=OP.is_lt)
            bnd = sb.tile([128, SPC], F32)
            nc.vector.tensor_tensor(bnd[:], f0[:], y[:], op=OP.is_equal)
            lf = sb.tile([128, SPC], F32)
            nc.vector.scalar_tensor_tensor(
                out=lf[:], in0=f0[:], scalar=-1.0, in1=lt[:], op0=OP.add, op1=OP.add
            )
            idx16 = sb.tile([128, SPC], I16)
            nc.vector.scalar_tensor_tensor(
                out=idx16[:], in0=bnd[:], scalar=float(ABASE), in1=lf[:],
                op0=OP.mult, op1=OP.add,
            )

            # ---- chunked gather + store ----
            out_v = out[:].rearrange("(p j) d -> p (j d)", p=128)
            for ci, (j0, jbc, q) in enumerate(CHUNKS):
                g = gp.tile([128, jbc * D], U16, tag=f"g{ci}")
                nc.gpsimd.dma_gather(
                    g[:].rearrange("p (j d) -> p j d", d=D),
                    tcb[0 : ABASE + V],
                    idx16[:, j0 * 8 : (j0 + jbc) * 8],
                    num_idxs=128 * jbc,
                    num_idxs_reg=nidx_regs[jbc],
                    elem_size=D,
                    single_packet=False,
                    queue_num=q,
                )
                eng = nc.sync if ci % 2 == 0 else nc.scalar
                eng.dma_start(out=out_v[:, j0 * D : (j0 + jbc) * D], in_=g[:])
    nc.compile()
    return nc


_NC = None


def _row_perm():
    """out row r holds gather position i(r); position i handles token
    t(i) = (i%16)*512 + i//16 (x wrapped [16,512] across partitions)."""
    r = np.arange(TPC)
    p, j = r // 64, r % 64
    i = j * 128 + p
    return (i % 16) * SPC + i // 16  # token index held at row r


def _f32_to_bf16_bits(a):
    bits = np.ascontiguousarray(a, dtype=np.float32).view(np.uint32)
    return (((bits + 0x7FFF + ((bits >> 16) & 1)) >> 16) & 0xFFFF).astype(np.uint16)


def _build_tc(t):
    tc = np.zeros((VEXT, D), dtype=np.float32)
    tc[0:V] = t
    ext = np.vstack([t, np.zeros((1, D), dtype=np.float32)])
    tc[ABASE : ABASE + V] = 0.5 * (ext[0:V] + ext[1 : V + 1])
    return _f32_to_bf16_bits(tc)


def kernel(x, time_embedding):
    global _NC
    x = np.ascontiguousarray(np.asarray(x, dtype=np.float32))
    t = np.ascontiguousarray(np.asarray(time_embedding, dtype=np.float32))
    tcb = _build_tc(t)
    xf = x.reshape(-1)
    in_maps = []
    for c in range(N_CORES):
        xc = xf[c * TPC : (c + 1) * TPC].reshape(16, SPC)
        in_maps.append({"xr": np.ascontiguousarray(np.tile(xc, (8, 1))), "tcb": tcb})

    if _NC is None:
        _NC = build()
    res = bass_utils.run_bass_kernel_spmd(_NC, in_maps, core_ids=list(range(N_CORES)))
    global _LAST_RES
    _LAST_RES = res

    tkn = _row_perm()
    outs = []
    for c in range(N_CORES):
        oc = np.asarray(res.results[c]["out"])  # [TPC, D] uint16 (bf16 bits)
        of = (oc.astype(np.uint32) << 16).view(np.float32)
        full = np.empty_like(of)
        full[tkn] = of
        outs.append(full)
    return np.concatenate(outs, axis=0).reshape(B, S, D)
